# revision 5
# baseline (speedup 1.0000x reference)
"""Trainium2 Bass kernel for EnhancedOFTQKVLayer.

Computes out[b,s,o] = x[b,s,:] @ filt[o,:]^T + bias[o] where
filt = [Wq @ BD(cayley(q_R)); Wk @ BD(cayley(k_R)); Wv @ BD(cayley(v_R))]
(BD = block-diagonal, cayley(A) = (I-S) inv(I+S+eps I), S = 0.5(A-A^T)).

Distribution: data-parallel — batch b (8 rows) sharded one per NeuronCore;
attn_weight / bias / rotation blocks replicated. Each core:
  1. Cayley via SPD Newton-Schulz on P = (1+eps)^2 I - S^2 (all iterates are
     polynomials in S^2, hence symmetric -> lhsT=operand works without
     transposes; explicit symmetrization kills roundoff-asymmetry growth).
     fp16 iterations + fp32 polish.
  2. filtT[h,o] built on-chip: per 128-block, Q_n^T @ W_n^T with W^T obtained
     by PE-transpose of weight tiles. Stored fp16.
  3. Main matmul in fp16 (fp32 PSUM accumulation): x row-tiles PE-transposed
     so the contraction dim sits on partitions; bias fused into PSUM eviction.
"""

import numpy as np

import concourse.bass as bass
import concourse.mybir as mybir
import concourse.tile as tile
from concourse import bacc
from concourse.bass import ts
from concourse.masks import make_identity
from concourse.bass_utils import run_bass_kernel_spmd

F32 = mybir.dt.float32
F16 = mybir.dt.float16

HIDDEN = 1024
OUT_DIM = 3 * HIDDEN
SEQ = 4096
P = 128
NBLK = 8                 # 128-blocks per hidden
NROT = 24                # 3 * NBLK rotation blocks
EPS = 1e-6
N_CORES = 8

# Newton-Schulz schedule (validated offline against the jax reference:
# final Q max rel err ~4e-5, end-to-end fp16 pipeline rel err ~4e-4).
NEWTON_F16 = 10
NEWTON_F32 = 2
SYM_FROM = 4             # symmetrize from this fp16 iteration on
C0 = 2.0 / 261.0         # 2/(lam_min + lam_max_bound), lam_max(P) <= ~250 < 260

M_TILES = SEQ // P       # 32
O_TILES = OUT_DIM // 512  # 6


def build_body(ctx, tc):
    nc = tc.nc

    x = nc.dram_tensor("x", [SEQ, HIDDEN], F32, kind="ExternalInput").ap()
    w = nc.dram_tensor("w", [OUT_DIM, HIDDEN], F32, kind="ExternalInput").ap()
    bias = nc.dram_tensor("bias", [OUT_DIM], F32, kind="ExternalInput").ap()
    rmat = nc.dram_tensor("rmat", [NROT, P, P], F32, kind="ExternalInput").ap()
    out = nc.dram_tensor("out", [SEQ, OUT_DIM], F32, kind="ExternalOutput").ap()

    sub = nc.vector.tensor_sub
    add = nc.vector.tensor_add
    smul = nc.vector.tensor_scalar_mul
    cp = nc.vector.tensor_copy

    # ---- persistent pools (allocated low so phase-scoped pools above them) ----
    const = ctx.enter_context(tc.tile_pool(name="const", bufs=1))
    ftp = ctx.enter_context(tc.tile_pool(name="ftp", bufs=1))
    xp = ctx.enter_context(tc.tile_pool(name="xp", bufs=3))
    obp = ctx.enter_context(tc.tile_pool(name="obp", bufs=2))
    ps_tp = ctx.enter_context(tc.tile_pool(name="ps_tp", bufs=2, space="PSUM"))

    ident32 = const.tile([P, P], F32)
    make_identity(nc, ident32)
    ident16 = const.tile([P, P], F16)
    cp(ident16[:], ident32[:])
    eI2 = const.tile([P, P], F32)       # (1+eps)^2 I
    smul(eI2[:], ident32[:], float((1.0 + EPS) ** 2))
    eI1 = const.tile([P, P], F32)       # (1+eps) I
    smul(eI1[:], ident32[:], float(1.0 + EPS))
    twoI = const.tile([P, P], F32)      # 2 I
    smul(twoI[:], ident32[:], 2.0)
    x0c = const.tile([P, P], F16)       # c0 I  (Newton init)
    smul(x0c[:], ident32[:], float(C0))

    bias_bc = const.tile([P, OUT_DIM], F32)
    nc.sync.dma_start(bias_bc[:1, :], bias.unsqueeze(0))
    nc.gpsimd.partition_broadcast(bias_bc[:], bias_bc[:1, :])

    # filtT chunk tiles: ft[k][c, o] = filtT[k*128+c, o], fp16
    ft = [ftp.tile([P, OUT_DIM], F16, tag=f"ft{k}", name=f"ft{k}") for k in range(NBLK)]

    # ---- phase A+B scoped pools ----
    with (
        tc.tile_pool(name="newton", bufs=6) as npool,
        tc.tile_pool(name="qpool", bufs=1) as qpool,
        tc.tile_pool(name="wpool", bufs=2) as wpool,
        tc.tile_pool(name="ps_a", bufs=4, space="PSUM") as ps_a,
    ):
        # ---------- Phase A: Newton-Cayley, 24 blocks ----------
        q_tiles = []
        for n in range(NROT):
            a = npool.tile([P, P], F32, tag="a")
            nc.sync.dma_start(a[:], rmat[n])
            at_ps = ps_tp.tile([P, P], F32, tag="tp")
            nc.tensor.transpose(at_ps[:], a[:], ident32[:])
            s = npool.tile([P, P], F32, tag="s")
            sub(s[:], a[:], at_ps[:])
            smul(s[:], s[:], 0.5)                       # S
            sn = npool.tile([P, P], F32, tag="sn")
            smul(sn[:], s[:], -1.0)                     # -S = S^T (lhsT of S@S)
            t_ps = ps_a.tile([P, P], F32, tag="mm")
            nc.tensor.matmul(t_ps[:], lhsT=sn[:], rhs=s[:], start=True, stop=True)
            p32 = npool.tile([P, P], F32, tag="p32")
            sub(p32[:], eI2[:], t_ps[:])                # P = (1+e)^2 I - S^2
            bt = npool.tile([P, P], F32, tag="bt")      # B^T = (1+e)I + (2+e)S + S^2
            nc.vector.tensor_scalar(bt[:], s[:], float(2.0 + EPS), None,
                                    mybir.AluOpType.mult)
            add(bt[:], bt[:], t_ps[:])
            add(bt[:], bt[:], eI1[:])
            p16 = npool.tile([P, P], F16, tag="p16")
            cp(p16[:], p32[:])

            xx = npool.tile([P, P], F16, tag="x")
            cp(xx[:], x0c[:])
            for i in range(NEWTON_F16):
                t1 = ps_a.tile([P, P], F32, tag="mm")
                nc.tensor.matmul(t1[:], lhsT=p16[:], rhs=xx[:], start=True, stop=True)
                u = npool.tile([P, P], F16, tag="u")
                sub(u[:], twoI[:], t1[:])               # U = 2I - P X
                x1 = ps_a.tile([P, P], F32, tag="mm")
                nc.tensor.matmul(x1[:], lhsT=xx[:], rhs=u[:], start=True, stop=True)
                xx = npool.tile([P, P], F16, tag="x")
                if i < SYM_FROM:
                    cp(xx[:], x1[:])
                else:
                    xc = npool.tile([P, P], F32, tag="xc")
                    cp(xc[:], x1[:])
                    xt_ps = ps_tp.tile([P, P], F32, tag="tp")
                    nc.tensor.transpose(xt_ps[:], xc[:], ident32[:])
                    xs = npool.tile([P, P], F32, tag="xs")
                    add(xs[:], xc[:], xt_ps[:])
                    nc.scalar.activation(xx[:], xs[:],
                                         mybir.ActivationFunctionType.Copy,
                                         scale=0.5)
            # fp32 polish
            xf = npool.tile([P, P], F32, tag="xf")
            cp(xf[:], xx[:])
            for i in range(NEWTON_F32):
                t1 = ps_a.tile([P, P], F32, tag="mm")
                nc.tensor.matmul(t1[:], lhsT=p32[:], rhs=xf[:], start=True, stop=True)
                uf = npool.tile([P, P], F32, tag="uf")
                sub(uf[:], twoI[:], t1[:])
                x1 = ps_a.tile([P, P], F32, tag="mm")
                nc.tensor.matmul(x1[:], lhsT=xf[:], rhs=uf[:], start=True, stop=True)
                xc = npool.tile([P, P], F32, tag="xc")
                cp(xc[:], x1[:])
                xt_ps = ps_tp.tile([P, P], F32, tag="tp")
                nc.tensor.transpose(xt_ps[:], xc[:], ident32[:])
                xf = npool.tile([P, P], F32, tag="xf")
                add(xf[:], xc[:], xt_ps[:])
                smul(xf[:], xf[:], 0.5)
            # Q = B @ X  (lhsT = B^T); stored fp16, [row b, col c] = lhsT layout
            q_ps = ps_a.tile([P, P], F32, tag="mm")
            nc.tensor.matmul(q_ps[:], lhsT=bt[:], rhs=xf[:], start=True, stop=True)
            q16 = qpool.tile([P, P], F16, tag=f"q{n}")
            cp(q16[:], q_ps[:])
            q_tiles.append(q16)

        # ---------- Phase B: W^T (streamed in 512-col groups) then filtT ----------
        for og in range(O_TILES):          # six 512-wide output column groups
            part = og // 2                 # which of q/k/v this group belongs to
            wts = wpool.tile([P, NBLK, 512], F16, tag="wts")
            for j in range(4):             # four 128-row W tiles per group
                ot = og * 4 + j
                wrow = wpool.tile([P, HIDDEN], F32, tag="wrow")
                nc.sync.dma_start(wrow[:], w[ts(ot, P), :])
                wrow16 = wpool.tile([P, HIDDEN], F16, tag="wrow16")
                cp(wrow16[:], wrow[:])
                for k in range(NBLK):
                    tp = ps_tp.tile([P, P], F16, tag="tp")
                    nc.tensor.transpose(tp[:], wrow16[:, ts(k, P)], ident16[:])
                    cp(wts[:, k, ts(j, P)], tp[:])
            for k in range(NBLK):
                f_ps = ps_a.tile([P, 512], F32, tag="mm")
                nc.tensor.matmul(f_ps[:], lhsT=q_tiles[part * NBLK + k][:],
                                 rhs=wts[:, k, :], start=True, stop=True)
                cp(ft[k][:, ts(og, 512)], f_ps[:])

    # ---------- Phase C: main matmul ----------
    with tc.tile_pool(name="ps_out", bufs=6, space="PSUM") as ps_out:
        for mt in range(M_TILES):
            xr = xp.tile([P, HIDDEN], F32, tag="xr")
            nc.sync.dma_start(xr[:], x[ts(mt, P), :])
            xr16 = xp.tile([P, HIDDEN], F16, tag="xr16")
            cp(xr16[:], xr[:])
            xt = xp.tile([P, NBLK, P], F16, tag="xt")
            for k in range(NBLK):
                tp = ps_tp.tile([P, P], F16, tag="tp")
                nc.tensor.transpose(tp[:], xr16[:, ts(k, P)], ident16[:])
                cp(xt[:, k, :], tp[:])
            psums = [ps_out.tile([P, 512], F32, tag="po", name=f"po{mt}_{i}") for i in range(O_TILES)]
            for k in range(NBLK):
                for o in range(O_TILES):
                    nc.tensor.matmul(psums[o][:], lhsT=xt[:, k, :],
                                     rhs=ft[k][:, ts(o, 512)],
                                     start=(k == 0), stop=(k == NBLK - 1))
            ob = obp.tile([P, OUT_DIM], F32, tag="ob")
            for o in range(O_TILES):
                add(ob[:, ts(o, 512)], psums[o][:], bias_bc[:, ts(o, 512)])
            nc.sync.dma_start(out[ts(mt, P), :], ob[:])


_CACHE = {}


def build():
    if "nc" in _CACHE:
        return _CACHE["nc"]
    import contextlib

    nc = bacc.Bacc("TRN2", target_bir_lowering=False, debug=False)
    with tile.TileContext(nc) as tc:
        with contextlib.ExitStack() as ctx:
            build_body(ctx, tc)
    nc.compile()
    _CACHE["nc"] = nc
    return nc


def make_in_maps(attn_weight, bias, x, q_R, k_R, v_R):
    rmat = np.ascontiguousarray(
        np.concatenate([q_R, k_R, v_R], axis=0), dtype=np.float32)
    w = np.ascontiguousarray(attn_weight, dtype=np.float32)
    b = np.ascontiguousarray(bias, dtype=np.float32)
    return [
        {"x": np.ascontiguousarray(x[c], dtype=np.float32),
         "w": w, "bias": b, "rmat": rmat}
        for c in range(N_CORES)
    ]


def kernel(attn_weight, bias, x, q_R, k_R, v_R, **run_kwargs):
    nc = build()
    in_maps = make_in_maps(attn_weight, bias, x, q_R, k_R, v_R)
    res = run_bass_kernel_spmd(nc, in_maps, core_ids=list(range(N_CORES)),
                               **run_kwargs)
    out = np.stack([res.results[c]["out"] for c in range(N_CORES)], axis=0)
    _CACHE["last_results"] = res
    return out


# revision 8
# speedup vs baseline: 1.5939x; 1.5939x over previous
"""Trainium2 Bass kernel for EnhancedOFTQKVLayer.

Computes out[b,s,o] = x[b,s,:] @ filt[o,:]^T + bias[o] where
filt = [Wq @ BD(cayley(q_R)); Wk @ BD(cayley(k_R)); Wv @ BD(cayley(v_R))]
(BD = block-diagonal, cayley(A) = (I-S) inv(I+S+eps I), S = 0.5(A-A^T)).

Distribution: data-parallel — batch b (8 rows) sharded one per NeuronCore;
attn_weight / bias / rotation blocks replicated. Each core:
  1. Cayley via SPD Newton-Schulz on P = (1+eps)^2 I - S^2 (all iterates are
     polynomials in S^2, hence symmetric -> lhsT=operand works without
     transposes; periodic symmetrization kills roundoff-asymmetry growth).
     fp16 iterations + fp32 polish, emitted iteration-major so the 24
     independent chains pipeline densely on the PE.
  2. filtT[h,o] built on-chip: per 128-block, Q_n^T @ W_n^T with W^T obtained
     by PE-transpose of weight tiles. Stored bf16.
  3. Main matmul in bf16 (2 cols/cycle on the PE; fp32 PSUM accumulation):
     x row-tiles PE-transposed so the contraction dim sits on partitions;
     bias pre-loaded into PSUM via a ones-matmul so eviction is a plain copy
     that can run on whichever of ACT/DVE is idle.
"""

import numpy as np

import concourse.bass as bass
import concourse.mybir as mybir
import concourse.tile as tile
from concourse import bacc
from concourse.bass import ts
from concourse.masks import make_identity
from concourse.bass_utils import run_bass_kernel_spmd

F32 = mybir.dt.float32
F16 = mybir.dt.float16
BF16 = mybir.dt.bfloat16

MAIN_DT = BF16           # dtype of the big matmul inputs (x, filtT)

HIDDEN = 1024
OUT_DIM = 3 * HIDDEN
SEQ = 4096
P = 128
NBLK = 8                 # 128-blocks per hidden
NROT = 24                # 3 * NBLK rotation blocks
EPS = 1e-6
N_CORES = 8

# Newton-Schulz schedule (validated offline against the jax reference).
NEWTON_F16 = 9
NEWTON_F32 = 2
SYM_ITERS = {4, 6, 8}    # symmetrize on these fp16 iterations
C0 = 2.0 / 261.0         # 2/(lam_min + lam_max_bound), lam_max(P) ~249 < 260

M_TILES = SEQ // P       # 32
O_TILES = OUT_DIM // 512  # 6


def build_body(ctx, tc):
    nc = tc.nc

    x = nc.dram_tensor("x", [SEQ, HIDDEN], F32, kind="ExternalInput").ap()
    w = nc.dram_tensor("w", [OUT_DIM, HIDDEN], F32, kind="ExternalInput").ap()
    bias = nc.dram_tensor("bias", [OUT_DIM], F32, kind="ExternalInput").ap()
    rmat = nc.dram_tensor("rmat", [NROT, P, P], F32, kind="ExternalInput").ap()
    out = nc.dram_tensor("out", [SEQ, OUT_DIM], F32, kind="ExternalOutput").ap()

    sub = nc.vector.tensor_sub
    add = nc.vector.tensor_add
    smul = nc.vector.tensor_scalar_mul
    cp = nc.vector.tensor_copy
    acp = nc.any.tensor_copy

    # ---- persistent pools ----
    const = ctx.enter_context(tc.tile_pool(name="const", bufs=1))
    ftp = ctx.enter_context(tc.tile_pool(name="ftp", bufs=1))
    xp = ctx.enter_context(tc.tile_pool(name="xp", bufs=3))
    obp = ctx.enter_context(tc.tile_pool(name="obp", bufs=4))
    ps_tp = ctx.enter_context(tc.tile_pool(name="ps_tp", bufs=2, space="PSUM"))

    ident32 = const.tile([P, P], F32)
    make_identity(nc, ident32)
    identb = const.tile([P, P], MAIN_DT)
    cp(identb[:], ident32[:])
    eI2 = const.tile([P, P], F32)       # (1+eps)^2 I
    smul(eI2[:], ident32[:], float((1.0 + EPS) ** 2))
    eI12 = const.tile([P, P], F32)      # ((1+eps) + (1+eps)^2) I
    smul(eI12[:], ident32[:], float((1.0 + EPS) + (1.0 + EPS) ** 2))
    twoI = const.tile([P, P], F32)      # 2 I
    smul(twoI[:], ident32[:], 2.0)
    x0c = const.tile([P, P], F16)       # c0 I  (Newton init)
    smul(x0c[:], ident32[:], float(C0))
    ones_b = const.tile([P, P], MAIN_DT)
    nc.vector.memset(ones_b[:], 1.0)

    # bias, broadcast via ones-matmul: row 0 = bias, other rows 0
    bias_sb = const.tile([P, OUT_DIM], MAIN_DT)
    nc.vector.memset(bias_sb[:], 0.0)
    with tc.tile_pool(name="biasld", bufs=1) as bl:
        bias_row = bl.tile([1, OUT_DIM], F32)
        nc.sync.dma_start(bias_row[:], bias.unsqueeze(0))
        cp(bias_sb[:1, :], bias_row[:])

    # filtT chunk tiles: ft[k][c, o] = filtT[k*128+c, o]
    ft = [ftp.tile([P, OUT_DIM], MAIN_DT, tag=f"ft{k}", name=f"ft{k}")
          for k in range(NBLK)]

    # ---- phase A+B scoped pools ----
    with (
        tc.tile_pool(name="nper", bufs=1) as nper,     # per-block persistents
        tc.tile_pool(name="nx", bufs=1) as nxp,        # per-block X (double buf)
        tc.tile_pool(name="nrot", bufs=5) as nrot,     # rotating temps
        tc.tile_pool(name="qpool", bufs=1) as qpool,
        tc.tile_pool(name="wpool", bufs=2) as wpool,
        tc.tile_pool(name="ps_a", bufs=6, space="PSUM") as ps_a,
    ):
        # ---------- Phase A: Newton-Cayley, 24 blocks, iteration-major ----------
        s_t, p32_t, p16_t, x_t = {}, {}, {}, {}
        for n in range(NROT):
            a = nrot.tile([P, P], F32, tag="a")
            nc.sync.dma_start(a[:], rmat[n])
            at_ps = ps_tp.tile([P, P], F32, tag="tp")
            nc.tensor.transpose(at_ps[:], a[:], ident32[:])
            s = nper.tile([P, P], F32, tag=f"s{n}", name=f"s{n}")
            sub(s[:], a[:], at_ps[:])
            smul(s[:], s[:], 0.5)                       # S
            sn = nrot.tile([P, P], F32, tag="sn")
            smul(sn[:], s[:], -1.0)                     # -S = S^T (lhsT of S@S)
            t_ps = ps_a.tile([P, P], F32, tag="mm")
            nc.tensor.matmul(t_ps[:], lhsT=sn[:], rhs=s[:], start=True, stop=True)
            p32 = nper.tile([P, P], F32, tag=f"p32{n}", name=f"p32{n}")
            sub(p32[:], eI2[:], t_ps[:])                # P = (1+e)^2 I - S^2
            p16 = nper.tile([P, P], F16, tag=f"p16{n}", name=f"p16{n}")
            acp(p16[:], p32[:])
            xx = nxp.tile([P, P], F16, tag=f"x{n}", name=f"x{n}_init")
            acp(xx[:], x0c[:])
            s_t[n], p32_t[n], p16_t[n], x_t[n] = s, p32, p16, xx

        for i in range(NEWTON_F16):
            do_sym = i in SYM_ITERS
            for n in range(NROT):
                t1 = ps_a.tile([P, P], F32, tag="mm")
                nc.tensor.matmul(t1[:], lhsT=p16_t[n][:], rhs=x_t[n][:],
                                 start=True, stop=True)
                u = nrot.tile([P, P], F16, tag="u")
                sub(u[:], twoI[:], t1[:])               # U = 2I - P X
                x1 = ps_a.tile([P, P], F32, tag="mm")
                nc.tensor.matmul(x1[:], lhsT=x_t[n][:], rhs=u[:],
                                 start=True, stop=True)
                xx = nxp.tile([P, P], F16, tag=f"x{n}", name=f"x{n}_{i}")
                if not do_sym:
                    cp(xx[:], x1[:])
                else:
                    xc = nrot.tile([P, P], F32, tag="xc")
                    cp(xc[:], x1[:])
                    xt_ps = ps_tp.tile([P, P], F32, tag="tp")
                    nc.tensor.transpose(xt_ps[:], xc[:], ident32[:])
                    xs = nrot.tile([P, P], F32, tag="xs")
                    add(xs[:], xc[:], xt_ps[:])
                    nc.scalar.activation(xx[:], xs[:],
                                         mybir.ActivationFunctionType.Copy,
                                         scale=0.5)
                x_t[n] = xx

        xf_t = {}
        for n in range(NROT):
            xf = nxp.tile([P, P], F32, tag=f"xf{n}", name=f"xf{n}_init")
            acp(xf[:], x_t[n][:])
            xf_t[n] = xf
        for i in range(NEWTON_F32):
            for n in range(NROT):
                t1 = ps_a.tile([P, P], F32, tag="mm")
                nc.tensor.matmul(t1[:], lhsT=p32_t[n][:], rhs=xf_t[n][:],
                                 start=True, stop=True)
                uf = nrot.tile([P, P], F32, tag="uf")
                sub(uf[:], twoI[:], t1[:])
                x1 = ps_a.tile([P, P], F32, tag="mm")
                nc.tensor.matmul(x1[:], lhsT=xf_t[n][:], rhs=uf[:],
                                 start=True, stop=True)
                xc = nrot.tile([P, P], F32, tag="xc")
                cp(xc[:], x1[:])
                xt_ps = ps_tp.tile([P, P], F32, tag="tp")
                nc.tensor.transpose(xt_ps[:], xc[:], ident32[:])
                xf = nxp.tile([P, P], F32, tag=f"xf{n}", name=f"xf{n}_{i}")
                add(xf[:], xc[:], xt_ps[:])
                smul(xf[:], xf[:], 0.5)
                xf_t[n] = xf

        # Q = B @ X with B^T = (1+e)I + (2+e)S + S^2 = eI12 + (2+e)S - P
        q_tiles = []
        for n in range(NROT):
            bt = nrot.tile([P, P], F32, tag="bt")
            nc.vector.tensor_scalar(bt[:], s_t[n][:], float(2.0 + EPS), None,
                                    mybir.AluOpType.mult)
            add(bt[:], bt[:], eI12[:])
            sub(bt[:], bt[:], p32_t[n][:])
            q_ps = ps_a.tile([P, P], F32, tag="mm")
            nc.tensor.matmul(q_ps[:], lhsT=bt[:], rhs=xf_t[n][:],
                             start=True, stop=True)
            q16 = qpool.tile([P, P], MAIN_DT, tag=f"q{n}")
            acp(q16[:], q_ps[:])
            q_tiles.append(q16)

        # ---------- Phase B: W^T (streamed in 512-col groups) then filtT ----------
        for og in range(O_TILES):          # six 512-wide output column groups
            part = og // 2                 # which of q/k/v this group belongs to
            wts = wpool.tile([P, NBLK, 512], MAIN_DT, tag="wts")
            for j in range(4):             # four 128-row W tiles per group
                ot = og * 4 + j
                wrow = wpool.tile([P, HIDDEN], F32, tag="wrow")
                nc.sync.dma_start(wrow[:], w[ts(ot, P), :])
                wrow16 = wpool.tile([P, HIDDEN], MAIN_DT, tag="wrow16")
                cp(wrow16[:], wrow[:])
                for k in range(NBLK):
                    tp = ps_tp.tile([P, P], MAIN_DT, tag="tp")
                    nc.tensor.transpose(tp[:], wrow16[:, ts(k, P)], identb[:])
                    acp(wts[:, k, ts(j, P)], tp[:])
            for k in range(NBLK):
                f_ps = ps_a.tile([P, 512], F32, tag="mm")
                nc.tensor.matmul(f_ps[:], lhsT=q_tiles[part * NBLK + k][:],
                                 rhs=wts[:, k, :], start=True, stop=True)
                acp(ft[k][:, ts(og, 512)], f_ps[:])

    # ---------- Phase C: main matmul ----------
    with tc.tile_pool(name="ps_out", bufs=6, space="PSUM") as ps_out:
        for mt in range(M_TILES):
            xr = xp.tile([P, HIDDEN], F32, tag="xr")
            nc.sync.dma_start(xr[:], x[ts(mt, P), :])
            xr16 = xp.tile([P, HIDDEN], MAIN_DT, tag="xr16")
            cp(xr16[:], xr[:])
            xt = xp.tile([P, NBLK, P], MAIN_DT, tag="xt")
            for k in range(NBLK):
                tp = ps_tp.tile([P, P], MAIN_DT, tag="tp")
                nc.tensor.transpose(tp[:], xr16[:, ts(k, P)], identb[:])
                acp(xt[:, k, :], tp[:])
            psums = [ps_out.tile([P, 512], F32, tag="po", name=f"po{mt}_{i}")
                     for i in range(O_TILES)]
            for o in range(O_TILES):       # pre-load bias into each psum bank
                nc.tensor.matmul(psums[o][:], lhsT=ones_b[:],
                                 rhs=bias_sb[:, ts(o, 512)],
                                 start=True, stop=False, skip_group_check=True)
            for k in range(NBLK):
                for o in range(O_TILES):
                    nc.tensor.matmul(psums[o][:], lhsT=xt[:, k, :],
                                     rhs=ft[k][:, ts(o, 512)],
                                     start=False, stop=(k == NBLK - 1),
                                     skip_group_check=True)
            for o in range(O_TILES):
                ob = obp.tile([P, 512], F32, tag="ob", name=f"ob{mt}_{o}")
                acp(ob[:], psums[o][:])
                nc.sync.dma_start(out[ts(mt, P), ts(o, 512)], ob[:])


_CACHE = {}


def build():
    if "nc" in _CACHE:
        return _CACHE["nc"]
    import contextlib

    nc = bacc.Bacc("TRN2", target_bir_lowering=False, debug=False)
    with tile.TileContext(nc) as tc:
        with contextlib.ExitStack() as ctx:
            build_body(ctx, tc)
    nc.compile()
    _CACHE["nc"] = nc
    return nc


def make_in_maps(attn_weight, bias, x, q_R, k_R, v_R):
    rmat = np.ascontiguousarray(
        np.concatenate([q_R, k_R, v_R], axis=0), dtype=np.float32)
    w = np.ascontiguousarray(attn_weight, dtype=np.float32)
    b = np.ascontiguousarray(bias, dtype=np.float32)
    return [
        {"x": np.ascontiguousarray(x[c], dtype=np.float32),
         "w": w, "bias": b, "rmat": rmat}
        for c in range(N_CORES)
    ]


def kernel(attn_weight, bias, x, q_R, k_R, v_R, **run_kwargs):
    nc = build()
    in_maps = make_in_maps(attn_weight, bias, x, q_R, k_R, v_R)
    res = run_bass_kernel_spmd(nc, in_maps, core_ids=list(range(N_CORES)),
                               **run_kwargs)
    out = np.stack([res.results[c]["out"] for c in range(N_CORES)], axis=0)
    _CACHE["last_results"] = res
    return out


# revision 10
# speedup vs baseline: 1.9340x; 1.2134x over previous
"""Trainium2 Bass kernel for EnhancedOFTQKVLayer.

Computes out[b,s,o] = x[b,s,:] @ filt[o,:]^T + bias[o] where
filt = [Wq @ BD(cayley(q_R)); Wk @ BD(cayley(k_R)); Wv @ BD(cayley(v_R))]
(BD = block-diagonal, cayley(A) = (I-S) inv(I+S+eps I), S = 0.5(A-A^T)).

Distribution: data-parallel — batch b (8 rows) sharded one per NeuronCore;
attn_weight / bias / rotation blocks replicated. Each core:
  1. Cayley via SPD Newton-Schulz on P = (1+eps)^2 I - S^2 (all iterates are
     polynomials in S^2, hence symmetric -> lhsT=operand works without
     transposes; periodic symmetrization kills roundoff-asymmetry growth).
     fp16 iterations + fp32 polish. Blocks are processed in sets of 4 whose
     matmuls share one PSUM bank, so every elementwise step is one wide DVE
     op over [128, 512] instead of four narrow ones; emission is
     iteration-major so the independent sets pipeline densely on the PE.
  2. filtT[h,o] built on-chip: per 128-block, Q_n^T @ W_n^T with W^T obtained
     by PE-transpose of weight tiles. Stored bf16 in 48 (k, o-group) chunks.
  3. Main matmul in bf16 (fp32 PSUM accumulation): x row-tiles PE-transposed
     so the contraction dim sits on partitions; bias fused into the PSUM
     eviction on the vector engine.
"""

import numpy as np

import concourse.bass as bass
import concourse.mybir as mybir
import concourse.tile as tile
from concourse import bacc
from concourse.bass import ts
from concourse.masks import make_identity
from concourse.bass_utils import run_bass_kernel_spmd

F32 = mybir.dt.float32
F16 = mybir.dt.float16
BF16 = mybir.dt.bfloat16

MAIN_DT = BF16           # dtype of the big matmul inputs (x, filtT)

HIDDEN = 1024
OUT_DIM = 3 * HIDDEN
SEQ = 4096
P = 128
NBLK = 8                 # 128-blocks per hidden
NROT = 24                # 3 * NBLK rotation blocks
EPS = 1e-6
N_CORES = 8

NSETS = 6                # Newton processes blocks in sets of 4
SETB = 4

# Newton-Schulz schedule (validated offline against the jax reference).
NEWTON_F16 = 9
NEWTON_F32 = 2
SYM_ITERS = {4, 6, 8}    # symmetrize on these fp16 iterations
C0 = 2.0 / 261.0         # 2/(lam_min + lam_max_bound), lam_max(P) ~249 < 260

M_TILES = SEQ // P       # 32
O_TILES = OUT_DIM // 512  # 6


def build_body(ctx, tc):
    nc = tc.nc

    x = nc.dram_tensor("x", [SEQ, HIDDEN], F32, kind="ExternalInput").ap()
    w = nc.dram_tensor("w", [OUT_DIM, HIDDEN], F32, kind="ExternalInput").ap()
    bias = nc.dram_tensor("bias", [OUT_DIM], F32, kind="ExternalInput").ap()
    rmat = nc.dram_tensor("rmat", [NROT, P, P], F32, kind="ExternalInput").ap()
    out = nc.dram_tensor("out", [SEQ, OUT_DIM], F32, kind="ExternalOutput").ap()

    sub = nc.vector.tensor_sub
    add = nc.vector.tensor_add
    smul = nc.vector.tensor_scalar_mul
    cp = nc.vector.tensor_copy
    acp = nc.any.tensor_copy

    def bc(t):  # broadcast a [P, P] constant over a set's middle dim
        return t[:].unsqueeze(1).to_broadcast([P, SETB, P])

    # ---- persistent pools ----
    const = ctx.enter_context(tc.tile_pool(name="const", bufs=1))
    ftp = ctx.enter_context(tc.tile_pool(name="ftp", bufs=1))
    xp = ctx.enter_context(tc.tile_pool(name="xp", bufs=3))
    obp = ctx.enter_context(tc.tile_pool(name="obp", bufs=2))
    ps_tp = ctx.enter_context(tc.tile_pool(name="ps_tp", bufs=2, space="PSUM"))

    ident32 = const.tile([P, P], F32)
    make_identity(nc, ident32)
    identb = const.tile([P, P], MAIN_DT)
    cp(identb[:], ident32[:])
    eI2 = const.tile([P, P], F32)       # (1+eps)^2 I
    smul(eI2[:], ident32[:], float((1.0 + EPS) ** 2))
    eI12 = const.tile([P, P], F32)      # ((1+eps) + (1+eps)^2) I
    smul(eI12[:], ident32[:], float((1.0 + EPS) + (1.0 + EPS) ** 2))
    twoI = const.tile([P, P], F32)      # 2 I
    smul(twoI[:], ident32[:], 2.0)
    x0c = const.tile([P, P], F16)       # c0 I  (Newton init)
    smul(x0c[:], ident32[:], float(C0))

    bias_bc = const.tile([P, OUT_DIM], F32)
    nc.sync.dma_start(bias_bc[:1, :], bias.unsqueeze(0))
    nc.gpsimd.partition_broadcast(bias_bc[:], bias_bc[:1, :])

    # filtT chunks: ft[k][og][c, o'] = filtT[k*128+c, og*512+o']
    ft = [[ftp.tile([P, 512], MAIN_DT, tag=f"ft{k}_{og}", name=f"ft{k}_{og}")
           for og in range(O_TILES)] for k in range(NBLK)]

    # ---- phase A+B scoped pools ----
    with (
        tc.tile_pool(name="nper", bufs=1) as nper,     # per-set persistents
        tc.tile_pool(name="nx", bufs=1) as nxp,        # per-set X iterates
        tc.tile_pool(name="nrot", bufs=2) as nrot,     # rotating temps
        tc.tile_pool(name="qpool", bufs=1) as qpool,
        tc.tile_pool(name="wpool", bufs=2) as wpool,
        tc.tile_pool(name="ps_g", bufs=4, space="PSUM") as ps_g,
    ):
        # ---------- Phase A: Newton-Cayley, 6 sets of 4 blocks ----------
        s_s, p32_s, p16_s, x_s = [], [], [], []
        for s in range(NSETS):
            n0 = s * SETB
            aset = nrot.tile([P, SETB, P], F32, tag="a")
            nc.sync.dma_start(aset[:],
                              rmat[n0:n0 + SETB].rearrange("n p f -> p n f"))
            tpg = ps_tp.tile([P, SETB, P], F32, tag="tp")
            for j in range(SETB):
                nc.tensor.transpose(tpg[:, j, :], aset[:, j, :], ident32[:])
            sset = nper.tile([P, SETB, P], F32, tag=f"s{s}", name=f"s{s}")
            sub(sset[:], aset[:], tpg[:])
            smul(sset[:], sset[:], 0.5)                  # S
            g = ps_g.tile([P, SETB, P], F32, tag="g")
            for j in range(SETB):                        # S^T @ S = -S^2
                nc.tensor.matmul(g[:, j, :], lhsT=sset[:, j, :],
                                 rhs=sset[:, j, :], start=True, stop=True)
            p32s = nper.tile([P, SETB, P], F32, tag=f"p32{s}", name=f"p32{s}")
            add(p32s[:], bc(eI2), g[:])                  # P = (1+e)^2 I - S^2
            p16s = nper.tile([P, SETB, P], F16, tag=f"p16{s}", name=f"p16{s}")
            acp(p16s[:], p32s[:])
            xset = nxp.tile([P, SETB, P], F16, tag=f"x{s}", name=f"x{s}_init")
            acp(xset[:], bc(x0c))
            s_s.append(sset)
            p32_s.append(p32s)
            p16_s.append(p16s)
            x_s.append(xset)

        for i in range(NEWTON_F16):
            do_sym = i in SYM_ITERS
            for s in range(NSETS):
                g1 = ps_g.tile([P, SETB, P], F32, tag="g")
                for j in range(SETB):
                    nc.tensor.matmul(g1[:, j, :], lhsT=p16_s[s][:, j, :],
                                     rhs=x_s[s][:, j, :], start=True, stop=True)
                u = nrot.tile([P, SETB, P], F16, tag="u")
                sub(u[:], bc(twoI), g1[:])               # U = 2I - P X
                g2 = ps_g.tile([P, SETB, P], F32, tag="g")
                for j in range(SETB):
                    nc.tensor.matmul(g2[:, j, :], lhsT=x_s[s][:, j, :],
                                     rhs=u[:, j, :], start=True, stop=True)
                xset = nxp.tile([P, SETB, P], F16, tag=f"x{s}",
                                name=f"x{s}_{i}")
                if not do_sym:
                    cp(xset[:], g2[:])
                else:
                    xc = nrot.tile([P, SETB, P], F32, tag="xc")
                    cp(xc[:], g2[:])
                    tpg = ps_tp.tile([P, SETB, P], F32, tag="tp")
                    for j in range(SETB):
                        nc.tensor.transpose(tpg[:, j, :], xc[:, j, :],
                                            ident32[:])
                    add(xc[:], xc[:], tpg[:])
                    nc.scalar.activation(xset[:], xc[:],
                                         mybir.ActivationFunctionType.Copy,
                                         scale=0.5)
                x_s[s] = xset

        xf_s = []
        for s in range(NSETS):
            xf = nxp.tile([P, SETB, P], F32, tag=f"xf{s}", name=f"xf{s}_init")
            acp(xf[:], x_s[s][:])
            xf_s.append(xf)
        for i in range(NEWTON_F32):
            for s in range(NSETS):
                g1 = ps_g.tile([P, SETB, P], F32, tag="g")
                for j in range(SETB):
                    nc.tensor.matmul(g1[:, j, :], lhsT=p32_s[s][:, j, :],
                                     rhs=xf_s[s][:, j, :], start=True,
                                     stop=True)
                uf = nrot.tile([P, SETB, P], F32, tag="uf")
                sub(uf[:], bc(twoI), g1[:])
                g2 = ps_g.tile([P, SETB, P], F32, tag="g")
                for j in range(SETB):
                    nc.tensor.matmul(g2[:, j, :], lhsT=xf_s[s][:, j, :],
                                     rhs=uf[:, j, :], start=True, stop=True)
                xc = nrot.tile([P, SETB, P], F32, tag="xc")
                cp(xc[:], g2[:])
                tpg = ps_tp.tile([P, SETB, P], F32, tag="tp")
                for j in range(SETB):
                    nc.tensor.transpose(tpg[:, j, :], xc[:, j, :], ident32[:])
                xf = nxp.tile([P, SETB, P], F32, tag=f"xf{s}",
                              name=f"xf{s}_{i}")
                add(xf[:], xc[:], tpg[:])
                smul(xf[:], xf[:], 0.5)
                xf_s[s] = xf

        # Q = B @ X with B^T = (1+e)I + (2+e)S + S^2 = eI12 + (2+e)S - P
        q_s = []
        for s in range(NSETS):
            bt = nrot.tile([P, SETB, P], F32, tag="bt")
            nc.vector.tensor_scalar(bt[:], s_s[s][:], float(2.0 + EPS), None,
                                    mybir.AluOpType.mult)
            add(bt[:], bt[:], bc(eI12))
            sub(bt[:], bt[:], p32_s[s][:])
            g = ps_g.tile([P, SETB, P], F32, tag="g")
            for j in range(SETB):
                nc.tensor.matmul(g[:, j, :], lhsT=bt[:, j, :],
                                 rhs=xf_s[s][:, j, :], start=True, stop=True)
            qset = qpool.tile([P, SETB, P], MAIN_DT, tag=f"q{s}", name=f"q{s}")
            acp(qset[:], g[:])
            q_s.append(qset)

        def q_lhsT(n):
            return q_s[n // SETB][:, n % SETB, :]

        # ---------- Phase B: W^T (streamed in 512-col groups) then filtT ----------
        for og in range(O_TILES):          # six 512-wide output column groups
            part = og // 2                 # which of q/k/v this group belongs to
            wts = wpool.tile([P, NBLK, 512], MAIN_DT, tag="wts")
            for j4 in range(4):            # four 128-row W tiles per group
                ot = og * 4 + j4
                wrow = wpool.tile([P, HIDDEN], F32, tag="wrow")
                nc.sync.dma_start(wrow[:], w[ts(ot, P), :])
                for kh in range(2):                     # two 4-block groups
                    tpg = ps_tp.tile([P, SETB, P], F32, tag="tp")
                    for k4 in range(SETB):
                        k = kh * SETB + k4
                        nc.tensor.transpose(tpg[:, k4, :], wrow[:, ts(k, P)],
                                            ident32[:])
                    acp(wts[:, ts(kh, SETB), ts(j4, P)], tpg[:])
            for k in range(NBLK):
                fg = ps_g.tile([P, 512], F32, tag="g")
                nc.tensor.matmul(fg[:], lhsT=q_lhsT(part * NBLK + k),
                                 rhs=wts[:, k, :], start=True, stop=True)
                acp(ft[k][og][:], fg[:])

    # ---------- Phase C: main matmul ----------
    with tc.tile_pool(name="ps_out", bufs=6, space="PSUM") as ps_out:
        for mt in range(M_TILES):
            xr = xp.tile([P, HIDDEN], F32, tag="xr")
            nc.sync.dma_start(xr[:], x[ts(mt, P), :])
            xr16 = xp.tile([P, HIDDEN], MAIN_DT, tag="xr16")
            cp(xr16[:], xr[:])
            xt = xp.tile([P, NBLK, P], MAIN_DT, tag="xt")
            tpg = ps_tp.tile([P, NBLK, P], MAIN_DT, tag="tp")
            for k in range(NBLK):
                nc.tensor.transpose(tpg[:, k, :], xr16[:, ts(k, P)], identb[:])
            acp(xt[:], tpg[:])
            psums = [ps_out.tile([P, 512], F32, tag="po", name=f"po{mt}_{i}")
                     for i in range(O_TILES)]
            for k in range(NBLK):
                for o in range(O_TILES):
                    nc.tensor.matmul(psums[o][:], lhsT=xt[:, k, :],
                                     rhs=ft[k][o][:],
                                     start=(k == 0), stop=(k == NBLK - 1))
            for o in range(O_TILES):
                ob = obp.tile([P, 512], F32, tag="ob", name=f"ob{mt}_{o}")
                add(ob[:], psums[o][:], bias_bc[:, ts(o, 512)])
                nc.sync.dma_start(out[ts(mt, P), ts(o, 512)], ob[:])


_CACHE = {}


def build():
    if "nc" in _CACHE:
        return _CACHE["nc"]
    import contextlib

    nc = bacc.Bacc("TRN2", target_bir_lowering=False, debug=False)
    with tile.TileContext(nc) as tc:
        with contextlib.ExitStack() as ctx:
            build_body(ctx, tc)
    nc.compile()
    _CACHE["nc"] = nc
    return nc


def make_in_maps(attn_weight, bias, x, q_R, k_R, v_R):
    rmat = np.ascontiguousarray(
        np.concatenate([q_R, k_R, v_R], axis=0), dtype=np.float32)
    w = np.ascontiguousarray(attn_weight, dtype=np.float32)
    b = np.ascontiguousarray(bias, dtype=np.float32)
    return [
        {"x": np.ascontiguousarray(x[c], dtype=np.float32),
         "w": w, "bias": b, "rmat": rmat}
        for c in range(N_CORES)
    ]


def kernel(attn_weight, bias, x, q_R, k_R, v_R, **run_kwargs):
    nc = build()
    in_maps = make_in_maps(attn_weight, bias, x, q_R, k_R, v_R)
    res = run_bass_kernel_spmd(nc, in_maps, core_ids=list(range(N_CORES)),
                               **run_kwargs)
    out = np.stack([res.results[c]["out"] for c in range(N_CORES)], axis=0)
    _CACHE["last_results"] = res
    return out


# revision 11
# speedup vs baseline: 2.1485x; 1.1109x over previous
"""Trainium2 Bass kernel for EnhancedOFTQKVLayer.

Computes out[b,s,o] = x[b,s,:] @ filt[o,:]^T + bias[o] where
filt = [Wq @ BD(cayley(q_R)); Wk @ BD(cayley(k_R)); Wv @ BD(cayley(v_R))]
(BD = block-diagonal, cayley(A) = (I-S) inv(I+S+eps I), S = 0.5(A-A^T)).

Distribution: data-parallel — batch b (8 rows) sharded one per NeuronCore;
attn_weight / bias / rotation blocks replicated. Each core:
  1. Cayley via SPD Newton-Schulz on P = (1+eps)^2 I - S^2 (all iterates are
     polynomials in S^2, hence symmetric -> lhsT=operand works without
     transposes; periodic symmetrization kills roundoff-asymmetry growth).
     fp16 iterations + fp32 polish. Blocks are processed in sets of 4 whose
     matmuls share one PSUM bank, so every elementwise step is one wide DVE
     op over [128, 512] instead of four narrow ones; emission is
     iteration-major so the independent sets pipeline densely on the PE.
  2. filtT[h,o] built on-chip: per 128-block, Q_n^T @ W_n^T with W^T obtained
     by PE-transpose of weight tiles. Stored bf16 in 48 (k, o-group) chunks.
  3. Main matmul in bf16 (fp32 PSUM accumulation): x row-tiles PE-transposed
     so the contraction dim sits on partitions; bias fused into the PSUM
     eviction on the vector engine.
"""

import numpy as np

import concourse.bass as bass
import concourse.mybir as mybir
import concourse.tile as tile
from concourse import bacc
from concourse.bass import ts
from concourse.masks import make_identity
from concourse.bass_utils import run_bass_kernel_spmd

F32 = mybir.dt.float32
F16 = mybir.dt.float16
BF16 = mybir.dt.bfloat16

MAIN_DT = BF16           # dtype of the big matmul inputs (x, filtT)

HIDDEN = 1024
OUT_DIM = 3 * HIDDEN
SEQ = 4096
P = 128
NBLK = 8                 # 128-blocks per hidden
NROT = 24                # 3 * NBLK rotation blocks
EPS = 1e-6
N_CORES = 8

NSETS = 6                # Newton processes blocks in sets of 4
SETB = 4

# Newton-Schulz schedule (validated offline against the jax reference).
NEWTON_F16 = 9
NEWTON_F32 = 2
SYM_ITERS = {4, 6, 8}    # symmetrize on these fp16 iterations
C0 = 2.0 / 261.0         # 2/(lam_min + lam_max_bound), lam_max(P) ~249 < 260

M_TILES = SEQ // P       # 32
O_TILES = OUT_DIM // 512  # 6


def build_body(ctx, tc):
    nc = tc.nc

    x = nc.dram_tensor("x", [SEQ, HIDDEN], F32, kind="ExternalInput").ap()
    w = nc.dram_tensor("w", [OUT_DIM, HIDDEN], F32, kind="ExternalInput").ap()
    bias = nc.dram_tensor("bias", [OUT_DIM], F32, kind="ExternalInput").ap()
    rmat = nc.dram_tensor("rmat", [NROT, P, P], F32, kind="ExternalInput").ap()
    out = nc.dram_tensor("out", [SEQ, OUT_DIM], F32, kind="ExternalOutput").ap()

    sub = nc.vector.tensor_sub
    add = nc.vector.tensor_add
    smul = nc.vector.tensor_scalar_mul
    cp = nc.vector.tensor_copy
    acp = nc.any.tensor_copy

    def bc(t):  # broadcast a [P, P] constant over a set's middle dim
        return t[:].unsqueeze(1).to_broadcast([P, SETB, P])

    # ---- persistent pools ----
    const = ctx.enter_context(tc.tile_pool(name="const", bufs=1))
    ftp = ctx.enter_context(tc.tile_pool(name="ftp", bufs=1))
    xp = ctx.enter_context(tc.tile_pool(name="xp", bufs=3))
    obp = ctx.enter_context(tc.tile_pool(name="obp", bufs=2))
    ps_xt = ctx.enter_context(tc.tile_pool(name="ps_xt", bufs=2, space="PSUM"))

    ident32 = const.tile([P, P], F32)
    make_identity(nc, ident32)
    identb = const.tile([P, P], MAIN_DT)
    cp(identb[:], ident32[:])
    eI2 = const.tile([P, P], F32)       # (1+eps)^2 I
    smul(eI2[:], ident32[:], float((1.0 + EPS) ** 2))
    eI12 = const.tile([P, P], F32)      # ((1+eps) + (1+eps)^2) I
    smul(eI12[:], ident32[:], float((1.0 + EPS) + (1.0 + EPS) ** 2))
    twoI = const.tile([P, P], F32)      # 2 I
    smul(twoI[:], ident32[:], 2.0)
    x0c = const.tile([P, P], F16)       # c0 I  (Newton init)
    smul(x0c[:], ident32[:], float(C0))
    two_eye16 = const.tile([P, P], F16)  # 2 I (fp16, Newton rhs)
    smul(two_eye16[:], ident32[:], 2.0)

    bias_bc = const.tile([P, OUT_DIM], F32)
    nc.sync.dma_start(bias_bc[:1, :], bias.unsqueeze(0))
    nc.gpsimd.partition_broadcast(bias_bc[:], bias_bc[:1, :])

    # filtT chunks: ft[k][og][c, o'] = filtT[k*128+c, og*512+o']
    ft = [[ftp.tile([P, 512], MAIN_DT, tag=f"ft{k}_{og}", name=f"ft{k}_{og}")
           for og in range(O_TILES)] for k in range(NBLK)]

    # ---- phase A+B scoped pools ----
    with (
        tc.tile_pool(name="nper", bufs=1) as nper,     # per-set persistents
        tc.tile_pool(name="nx", bufs=1) as nxp,        # per-set X iterates
        tc.tile_pool(name="nrot", bufs=2) as nrot,     # rotating temps
        tc.tile_pool(name="qpool", bufs=1) as qpool,
        tc.tile_pool(name="wpool", bufs=2) as wpool,
        tc.tile_pool(name="wtsp", bufs=1) as wtsp,
        tc.tile_pool(name="ps_g", bufs=4, space="PSUM") as ps_g,
        tc.tile_pool(name="ps_tp", bufs=2, space="PSUM") as ps_tp,
    ):
        # x-tile prep shared by prefetch (below) and the main loop
        def emit_xprep(mt):
            xr = xp.tile([P, HIDDEN], F32, tag="xr", name=f"xr{mt}")
            nc.sync.dma_start(xr[:], x[ts(mt, P), :])
            xr16 = xp.tile([P, HIDDEN], MAIN_DT, tag="xr16", name=f"xr16_{mt}")
            cp(xr16[:], xr[:])
            xt = xp.tile([P, NBLK, P], MAIN_DT, tag="xt", name=f"xt{mt}")
            tpg = ps_xt.tile([P, NBLK, P], MAIN_DT, tag="xtp",
                             name=f"xtp{mt}")
            for k in range(NBLK):
                nc.tensor.transpose(tpg[:, k, :], xr16[:, ts(k, P)],
                                    identb[:])
            acp(xt[:], tpg[:])
            return xt
        # ---------- Phase A: Newton-Cayley, 6 sets of 4 blocks ----------
        s_s, p32_s, p16_s, x_s = [], [], [], []
        for s in range(NSETS):
            n0 = s * SETB
            aset = nrot.tile([P, SETB, P], F32, tag="a")
            nc.sync.dma_start(aset[:],
                              rmat[n0:n0 + SETB].rearrange("n p f -> p n f"))
            tpg = ps_tp.tile([P, SETB, P], F32, tag="tp")
            for j in range(SETB):
                nc.tensor.transpose(tpg[:, j, :], aset[:, j, :], ident32[:])
            sset = nper.tile([P, SETB, P], F32, tag=f"s{s}", name=f"s{s}")
            sub(sset[:], aset[:], tpg[:])
            smul(sset[:], sset[:], 0.5)                  # S
            g = ps_g.tile([P, SETB, P], F32, tag="g")
            for j in range(SETB):                        # S^T @ S = -S^2
                nc.tensor.matmul(g[:, j, :], lhsT=sset[:, j, :],
                                 rhs=sset[:, j, :], start=True, stop=True)
            p32s = nper.tile([P, SETB, P], F32, tag=f"p32{s}", name=f"p32{s}")
            add(p32s[:], bc(eI2), g[:])                  # P = (1+e)^2 I - S^2
            p16s = nper.tile([P, SETB, P], F16, tag=f"p16{s}", name=f"p16{s}")
            acp(p16s[:], p32s[:])
            xset = nxp.tile([P, SETB, P], F16, tag=f"x{s}", name=f"x{s}_init")
            acp(xset[:], bc(x0c))
            s_s.append(sset)
            p32_s.append(p32s)
            p16_s.append(p16s)
            x_s.append(xset)

        PREFETCH = 8
        xt_pre = [emit_xprep(mt) for mt in range(PREFETCH)]

        for i in range(NEWTON_F16):
            do_sym = i in SYM_ITERS
            for s in range(NSETS):
                g1 = ps_g.tile([P, SETB, P], F32, tag="g")
                for j in range(SETB):
                    nc.tensor.matmul(g1[:, j, :], lhsT=p16_s[s][:, j, :],
                                     rhs=x_s[s][:, j, :], start=True, stop=True)
                t1n = nrot.tile([P, SETB, P], F16, tag="t1n")
                nc.scalar.activation(t1n[:], g1[:],      # -T1, off the DVE
                                     mybir.ActivationFunctionType.Copy,
                                     scale=-1.0)
                g2 = ps_g.tile([P, SETB, P], F32, tag="g")
                for j in range(SETB):                    # X' = X(2I) - X T1
                    nc.tensor.matmul(g2[:, j, :], lhsT=x_s[s][:, j, :],
                                     rhs=two_eye16[:], start=True, stop=False)
                    nc.tensor.matmul(g2[:, j, :], lhsT=x_s[s][:, j, :],
                                     rhs=t1n[:, j, :], start=False, stop=True)
                xset = nxp.tile([P, SETB, P], F16, tag=f"x{s}",
                                name=f"x{s}_{i}")
                if not do_sym:
                    acp(xset[:], g2[:])
                else:
                    xc = nrot.tile([P, SETB, P], F32, tag="xc")
                    cp(xc[:], g2[:])
                    tpg = ps_tp.tile([P, SETB, P], F32, tag="tp")
                    for j in range(SETB):
                        nc.tensor.transpose(tpg[:, j, :], xc[:, j, :],
                                            ident32[:])
                    add(xc[:], xc[:], tpg[:])
                    nc.scalar.activation(xset[:], xc[:],
                                         mybir.ActivationFunctionType.Copy,
                                         scale=0.5)
                x_s[s] = xset

        xf_s = []
        for s in range(NSETS):
            xf = nxp.tile([P, SETB, P], F32, tag=f"xf{s}", name=f"xf{s}_init")
            acp(xf[:], x_s[s][:])
            xf_s.append(xf)
        for i in range(NEWTON_F32):
            for s in range(NSETS):
                g1 = ps_g.tile([P, SETB, P], F32, tag="g")
                for j in range(SETB):
                    nc.tensor.matmul(g1[:, j, :], lhsT=p32_s[s][:, j, :],
                                     rhs=xf_s[s][:, j, :], start=True,
                                     stop=True)
                uf = nrot.tile([P, SETB, P], F32, tag="uf")
                sub(uf[:], bc(twoI), g1[:])
                g2 = ps_g.tile([P, SETB, P], F32, tag="g")
                for j in range(SETB):
                    nc.tensor.matmul(g2[:, j, :], lhsT=xf_s[s][:, j, :],
                                     rhs=uf[:, j, :], start=True, stop=True)
                xf = nxp.tile([P, SETB, P], F32, tag=f"xf{s}",
                              name=f"xf{s}_{i}")
                acp(xf[:], g2[:])
                xf_s[s] = xf

        # Q = B @ X with B^T = (1+e)I + (2+e)S + S^2 = eI12 + (2+e)S - P
        q_s = []
        for s in range(NSETS):
            bt = nrot.tile([P, SETB, P], F32, tag="bt")
            nc.vector.tensor_scalar(bt[:], s_s[s][:], float(2.0 + EPS), None,
                                    mybir.AluOpType.mult)
            add(bt[:], bt[:], bc(eI12))
            sub(bt[:], bt[:], p32_s[s][:])
            g = ps_g.tile([P, SETB, P], F32, tag="g")
            for j in range(SETB):
                nc.tensor.matmul(g[:, j, :], lhsT=bt[:, j, :],
                                 rhs=xf_s[s][:, j, :], start=True, stop=True)
            qset = qpool.tile([P, SETB, P], MAIN_DT, tag=f"q{s}", name=f"q{s}")
            acp(qset[:], g[:])
            q_s.append(qset)

        def q_lhsT(n):
            return q_s[n // SETB][:, n % SETB, :]

        # ---------- Phase B: W^T (streamed in 512-col groups) then filtT ----------
        for og in range(O_TILES):          # six 512-wide output column groups
            part = og // 2                 # which of q/k/v this group belongs to
            wts = wtsp.tile([P, NBLK, 512], MAIN_DT, tag="wts")
            for j4 in range(4):            # four 128-row W tiles per group
                ot = og * 4 + j4
                wrow = wpool.tile([P, HIDDEN], F32, tag="wrow")
                nc.sync.dma_start(wrow[:], w[ts(ot, P), :])
                for kh in range(2):                     # two 4-block groups
                    tpg = ps_tp.tile([P, SETB, P], F32, tag="tp")
                    for k4 in range(SETB):
                        k = kh * SETB + k4
                        nc.tensor.transpose(tpg[:, k4, :], wrow[:, ts(k, P)],
                                            ident32[:])
                    acp(wts[:, ts(kh, SETB), ts(j4, P)], tpg[:])
            for k in range(NBLK):
                fg = ps_g.tile([P, 512], F32, tag="g")
                nc.tensor.matmul(fg[:], lhsT=q_lhsT(part * NBLK + k),
                                 rhs=wts[:, k, :], start=True, stop=True)
                acp(ft[k][og][:], fg[:])

        _CACHE["xt_pre"] = xt_pre

    # ---------- Phase C: main matmul ----------
    xt_pre = _CACHE.pop("xt_pre")
    with tc.tile_pool(name="ps_out", bufs=6, space="PSUM") as ps_out:
        for mt in range(M_TILES):
            xt = xt_pre[mt] if mt < len(xt_pre) else emit_xprep(mt)
            psums = [ps_out.tile([P, 512], F32, tag="po", name=f"po{mt}_{i}")
                     for i in range(O_TILES)]
            for k in range(NBLK):
                for o in range(O_TILES):
                    nc.tensor.matmul(psums[o][:], lhsT=xt[:, k, :],
                                     rhs=ft[k][o][:],
                                     start=(k == 0), stop=(k == NBLK - 1))
            for o in range(O_TILES):
                ob = obp.tile([P, 512], F32, tag="ob", name=f"ob{mt}_{o}")
                add(ob[:], psums[o][:], bias_bc[:, ts(o, 512)])
                nc.sync.dma_start(out[ts(mt, P), ts(o, 512)], ob[:])


_CACHE = {}


def build():
    if "nc" in _CACHE:
        return _CACHE["nc"]
    import contextlib

    nc = bacc.Bacc("TRN2", target_bir_lowering=False, debug=False)
    with tile.TileContext(nc) as tc:
        with contextlib.ExitStack() as ctx:
            build_body(ctx, tc)
    nc.compile()
    _CACHE["nc"] = nc
    return nc


def make_in_maps(attn_weight, bias, x, q_R, k_R, v_R):
    rmat = np.ascontiguousarray(
        np.concatenate([q_R, k_R, v_R], axis=0), dtype=np.float32)
    w = np.ascontiguousarray(attn_weight, dtype=np.float32)
    b = np.ascontiguousarray(bias, dtype=np.float32)
    return [
        {"x": np.ascontiguousarray(x[c], dtype=np.float32),
         "w": w, "bias": b, "rmat": rmat}
        for c in range(N_CORES)
    ]


def kernel(attn_weight, bias, x, q_R, k_R, v_R, **run_kwargs):
    nc = build()
    in_maps = make_in_maps(attn_weight, bias, x, q_R, k_R, v_R)
    res = run_bass_kernel_spmd(nc, in_maps, core_ids=list(range(N_CORES)),
                               **run_kwargs)
    out = np.stack([res.results[c]["out"] for c in range(N_CORES)], axis=0)
    _CACHE["last_results"] = res
    return out
